# revision 30
# baseline (speedup 1.0000x reference)
"""Multi-head causal attention (B=4, S=2048, E=1024, H=16, D=64) on 8 TRN2 cores.

Sharding: core c handles batch c//2 and head-half c%2 (8 heads). Each core
computes Q/K/V projections, causal attention for its heads, and a partial
output projection over its heads; a 2-core bf16 AllGather per 256-token
chunk exchanges the pair's partials, and both cores sum them locally, so
every core holds the full output for its batch.

Layouts are transposed (feature-on-partition): the host supplies X^T and
head-packed weights so no on-chip transposes are needed. Attention runs in
S^T = K.Q^T layout (keys on partitions); softmax denominators come from a
ones-column appended to V so the PV matmul emits them for free. Scores for
a head pair go side by side into one 2-bank PSUM tile so a single ScalarE
exp covers both. The V bias and output bias are folded on the host into a
single bias row that enters the output projection as a K=1 matmul. Q/K
projections for later head pairs are interleaved with qt=0 attention so the
TensorE and ScalarE pipelines overlap from the start.
"""

import sys

sys.path.insert(0, "/opt/trn_rl_repo")

import numpy as np
import ml_dtypes

import concourse.bass as bass
import concourse.bacc as bacc
import concourse.tile as tile
import concourse.mybir as mybir
import concourse.bass_utils as bass_utils

B, S, E, H, D = 4, 2048, 1024, 16, 64
N_CORES = 8
HPC = H // 2          # heads per core
NPAIR = HPC // 2      # head pairs per core
SQ = 512              # q tile width
SK = 128              # k tile width
NQT = S // SQ         # 4
NKT = S // SK         # 16
NE = E // 128         # 8 contraction tiles
NCH = 8               # output exchange chunks (256 tokens each)
F32 = mybir.dt.float32
F32R = mybir.dt.float32r
BF16 = mybir.dt.bfloat16
BF16_NP = ml_dtypes.bfloat16
# bf16 CCE ReduceScatter faults the exec unit; AllGather is bypass (no CCE
# math) so bf16 works: gather both partials, then sum them locally.
CC_DT = mybir.dt.bfloat16

REPLICA_GROUPS = [[0, 1], [2, 3], [4, 5], [6, 7]]
AF = mybir.ActivationFunctionType
ALU = mybir.AluOpType


def build_kernel():
    nc = bacc.Bacc("TRN2", target_bir_lowering=False, debug=False,
                   num_devices=N_CORES)

    xt_d = nc.dram_tensor("XT", [E, S], BF16, kind="ExternalInput")
    wq_d = nc.dram_tensor("Wq", [E, HPC * D], BF16, kind="ExternalInput")
    wk_d = nc.dram_tensor("Wk", [E, HPC * D], BF16, kind="ExternalInput")
    wv_d = nc.dram_tensor("Wv", [E, HPC * D], BF16, kind="ExternalInput")
    bq_d = nc.dram_tensor("bq", [1, HPC * D], BF16, kind="ExternalInput")
    bk_d = nc.dram_tensor("bk", [1, HPC * D], BF16, kind="ExternalInput")
    wo_d = nc.dram_tensor("Wo", [HPC * D, E], BF16, kind="ExternalInput")
    bo2_d = nc.dram_tensor("bo2", [1, E], BF16, kind="ExternalInput")
    ones_d = nc.dram_tensor("ones", [1, 128], F32R, kind="ExternalInput")
    onesb_d = nc.dram_tensor("onesb", [1, SQ], BF16, kind="ExternalInput")
    mask_d = nc.dram_tensor("masks", [4, SK, 2 * SQ], BF16, kind="ExternalInput")
    out_d = nc.dram_tensor("out", [S, E], F32, kind="ExternalOutput")

    with tile.TileContext(nc) as tc:
        with (
            tc.tile_pool(name="persist", bufs=1) as persist,
            tc.tile_pool(name="dram", bufs=1, space="DRAM") as dram,
            tc.tile_pool(name="p1_in", bufs=1) as p1_in,
            tc.tile_pool(name="mm_ps", bufs=2, space="PSUM") as mm_ps,
            tc.tile_pool(name="st_ps", bufs=2, space="PSUM") as st_ps,
            tc.tile_pool(name="pv_ps", bufs=1, space="PSUM") as pv_ps,
            tc.tile_pool(name="probs", bufs=4) as probs_pool,
            tc.tile_pool(name="norm", bufs=2) as norm_pool,
            tc.tile_pool(name="op_sb", bufs=2) as op_sb,
        ):
            # ---- persistent SBUF tensors ----
            qt_sb = [persist.tile([128, S], BF16, tag=f"qt{p}", name=f"qt{p}")
                     for p in range(NPAIR)]
            kt_sb = [persist.tile([128, S], BF16, tag=f"kt{p}", name=f"kt{p}")
                     for p in range(NPAIR)]
            v_sb = [persist.tile([128, HPC, D + 1], BF16, tag=f"v{st}", name=f"v{st}")
                    for st in range(NKT)]
            ao_sb = [persist.tile([128, S], BF16, tag=f"ao{p}", name=f"ao{p}")
                     for p in range(NPAIR)]
            wo_sb = [persist.tile([128, E], BF16, tag=f"wo{p}", name=f"wo{p}")
                     for p in range(NPAIR)]
            bq_sb = persist.tile([1, HPC * D], BF16, tag="bq")
            bk_sb = persist.tile([1, HPC * D], BF16, tag="bk")
            bo2_sb = persist.tile([1, E], BF16, tag="bo2")
            ones_sb = persist.tile([1, 128], F32R, tag="ones")
            onesb_sb = persist.tile([1, SQ], BF16, tag="onesb")
            mask_sb = [persist.tile([SK, 2 * SQ], BF16, tag=f"mask{oi}",
                                    name=f"mask{oi}") for oi in range(4)]
            cc_in = [dram.tile([256, E], CC_DT, name=f"cc_in{c}")
                     for c in range(NCH)]
            cc_out = [dram.tile([512, E], CC_DT, name=f"cc_out{c}")
                      for c in range(NCH)]

            xt_sb = [p1_in.tile([128, S], BF16, tag=f"xt{e}", name=f"xt{e}")
                     for e in range(NE)]
            wq_sb = [p1_in.tile([128, HPC * D], BF16, tag=f"wq{e}", name=f"wq{e}")
                     for e in range(NE)]
            wk_sb = [p1_in.tile([128, HPC * D], BF16, tag=f"wk{e}", name=f"wk{e}")
                     for e in range(NE)]
            wv_sb = [p1_in.tile([128, HPC * D], BF16, tag=f"wv{e}", name=f"wv{e}")
                     for e in range(NE)]
            # DMA order = consumption order
            for e in range(NE):
                nc.sync.dma_start(xt_sb[e][:], xt_d[e * 128:(e + 1) * 128, :])
                nc.sync.dma_start(wv_sb[e][:], wv_d[e * 128:(e + 1) * 128, :])
                nc.sync.dma_start(wq_sb[e][:], wq_d[e * 128:(e + 1) * 128, :])
                nc.sync.dma_start(wk_sb[e][:], wk_d[e * 128:(e + 1) * 128, :])
            nc.sync.dma_start(bq_sb[:], bq_d[:])
            nc.sync.dma_start(bk_sb[:], bk_d[:])
            nc.sync.dma_start(bo2_sb[:], bo2_d[:])
            nc.sync.dma_start(ones_sb[:], ones_d[:])
            nc.sync.dma_start(onesb_sb[:], onesb_d[:])
            for oi in range(4):
                nc.sync.dma_start(mask_sb[oi][:], mask_d[oi])
            for p in range(NPAIR):
                nc.sync.dma_start(wo_sb[p][:], wo_d[p * 128:(p + 1) * 128, :])

            def emit_v(st):
                ps = mm_ps.tile([128, HPC * D], F32, tag="mm", name="psv")
                for e in range(NE):
                    nc.tensor.matmul(
                        ps[:],
                        xt_sb[e][:, st * 128:(st + 1) * 128],
                        wv_sb[e][:],
                        start=(e == 0), stop=(e == NE - 1),
                    )
                nc.scalar.copy(
                    v_sb[st][:, :, 0:D],
                    ps[:].rearrange("p (h d) -> p h d", h=HPC),
                )
                nc.vector.memset(v_sb[st][:, :, D:D + 1], 1.0)

            def emit_qk(p):
                for st in range(NQT):
                    for w_sb, b_sb, o_sb in (
                        (wq_sb, bq_sb, qt_sb),
                        (wk_sb, bk_sb, kt_sb),
                    ):
                        ps = mm_ps.tile([128, SQ], F32, tag="mm", name="ps")
                        for e in range(NE):
                            nc.tensor.matmul(
                                ps[:],
                                w_sb[e][:, p * 128:(p + 1) * 128],
                                xt_sb[e][:, st * SQ:(st + 1) * SQ],
                                start=(e == 0), stop=False,
                            )
                        nc.tensor.matmul(  # + bias via K=1 ones row
                            ps[:], b_sb[0:1, p * 128:(p + 1) * 128],
                            onesb_sb[0:1, :], start=False, stop=True,
                        )
                        nc.scalar.copy(o_sb[p][:, st * SQ:(st + 1) * SQ], ps[:])

            def emit_attn(qt, p):
                nkt_q = 4 * (qt + 1)  # causal: kt*128 <= qt*512+511
                qs = slice(qt * SQ, (qt + 1) * SQ)
                pv = [pv_ps.tile([128, SQ], F32, tag=f"pv{u}", name=f"pv{u}")
                      for u in range(2)]
                for kt in range(nkt_q):
                    ks = slice(kt * SK, (kt + 1) * SK)
                    sps = st_ps.tile([128, 2 * SQ], F32, tag="st", name="st")
                    for u in range(2):
                        nc.tensor.matmul(
                            sps[:, u * SQ:(u + 1) * SQ],
                            kt_sb[p][u * D:(u + 1) * D, ks],
                            qt_sb[p][u * D:(u + 1) * D, qs],
                            start=True, stop=True,
                        )
                    pt = probs_pool.tile([128, 2 * SQ], BF16, tag="pt", name="pt")
                    oi = kt - 4 * qt
                    if oi >= 0:  # diagonal block: exp then mask both halves
                        et = probs_pool.tile([128, 2 * SQ], BF16, tag="et",
                                             name="et")
                        nc.scalar.activation(et[:], sps[:], AF.Exp, scale=0.125)
                        nc.vector.tensor_tensor(pt[:], et[:], mask_sb[oi][:],
                                                ALU.mult)
                    else:
                        nc.scalar.activation(pt[:], sps[:], AF.Exp, scale=0.125)
                    for u in range(2):
                        nc.tensor.matmul(
                            pv[u][0:D + 1, :],
                            v_sb[kt][:, 2 * p + u, :],
                            pt[:, u * SQ:(u + 1) * SQ],
                            start=(kt == 0), stop=(kt == nkt_q - 1),
                        )
                # copy out + denom to SBUF first, freeing both pv banks early
                pvs, den = [], []
                for u in range(2):
                    pvs.append(norm_pool.tile([D, SQ], F32, tag=f"pvs{u}",
                                              name=f"pvs{u}"))
                    nc.vector.tensor_copy(pvs[u][:], pv[u][0:D, :])
                    den.append(norm_pool.tile([1, SQ], F32, tag=f"den{u}",
                                              name=f"den{u}"))
                    nc.vector.tensor_copy(den[u][:], pv[u][D:D + 1, :])
                for u in range(2):
                    lo, hi = u * D, (u + 1) * D
                    rcp = norm_pool.tile([1, SQ], F32, tag="rcp", name="rcp")
                    nc.vector.reciprocal_approx_fast(rcp[:], den[u][:])
                    rcp_r = norm_pool.tile([1, SQ], F32R, tag="rcp_r", name="rcp_r")
                    nc.vector.tensor_copy(rcp_r[:], rcp[:])
                    # broadcast 1/den across the d partitions
                    bc = mm_ps.tile([D, SQ], F32, tag="mm", name=f"bc{u}")
                    nc.tensor.matmul(bc[:], ones_sb[0:1, 0:D], rcp_r[:],
                                     start=True, stop=True)
                    nc.vector.tensor_tensor(
                        ao_sb[p][lo:hi, qs], pvs[u][:], bc[:], ALU.mult,
                    )

            def emit_chunk(cq):
                # output projection + exchange for 256 tokens
                for sst in range(2):
                    stg = cq * 2 + sst
                    ss = slice(stg * 128, (stg + 1) * 128)
                    for et in range(2):
                        es = slice(et * 512, (et + 1) * 512)
                        ps = mm_ps.tile([128, 512], F32, tag="mm", name="op")
                        for p2 in range(NPAIR):
                            nc.tensor.matmul(
                                ps[:], ao_sb[p2][:, ss], wo_sb[p2][:, es],
                                start=(p2 == 0), stop=False,
                            )
                        nc.tensor.matmul(  # + (bo/2 + bv@Wo) via K=1 ones row
                            ps[:], onesb_sb[0:1, 0:128], bo2_sb[0:1, es],
                            start=False, stop=True,
                        )
                        ob = op_sb.tile([128, 512], CC_DT, tag="ob", name="ob")
                        nc.vector.tensor_copy(ob[:], ps[:])
                        nc.sync.dma_start(cc_in[cq][sst * 128:(sst + 1) * 128, es],
                                          ob[:])
                nc.gpsimd.collective_compute(
                    "AllGather", ALU.bypass,
                    ins=[cc_in[cq][:].opt()],
                    outs=[cc_out[cq][:].opt()],
                    replica_groups=REPLICA_GROUPS,
                )
                # rows 0-255 = rank0's partial, 256-511 = rank1's; both cores
                # materialize the full sum for the 256 tokens.
                for sst2 in range(2):
                    ra = op_sb.tile([128, E], CC_DT, tag="ra", name="ra")
                    rb = op_sb.tile([128, E], CC_DT, tag="rb", name="rb")
                    nc.sync.dma_start(
                        ra[:], cc_out[cq][sst2 * 128:(sst2 + 1) * 128, :])
                    nc.sync.dma_start(
                        rb[:], cc_out[cq][256 + sst2 * 128:256 + (sst2 + 1) * 128, :])
                    sm = op_sb.tile([128, E], F32, tag="sm", name="sm")
                    nc.vector.tensor_tensor(sm[:], ra[:], rb[:], ALU.add)
                    nc.sync.dma_start(
                        out_d[cq * 256 + sst2 * 128:cq * 256 + (sst2 + 1) * 128, :],
                        sm[:])

            # ---- schedule ----
            for st in range(4):
                emit_v(st)
            emit_qk(0)
            for st in range(4, NKT):
                emit_v(st)
            # interleave remaining projections with qt=0 attention
            for p in range(NPAIR):
                if p + 1 < NPAIR:
                    emit_qk(p + 1)
                emit_attn(0, p)
            emit_chunk(0)
            emit_chunk(1)
            for qt in range(1, NQT):
                for p in range(NPAIR):
                    emit_attn(qt, p)
                emit_chunk(2 * qt)
                emit_chunk(2 * qt + 1)

    nc.compile()
    return nc


_NC_CACHE = None


def get_nc():
    global _NC_CACHE
    if _NC_CACHE is None:
        _NC_CACHE = build_kernel()
    return _NC_CACHE


def make_in_maps(X, Wq, Wk, Wv, bq, bk, bv, Wo, bo):
    X = np.asarray(X, np.float32)
    Wq, Wk, Wv = (np.asarray(w, np.float32) for w in (Wq, Wk, Wv))
    bq, bk, bv = (np.asarray(b, np.float32) for b in (bq, bk, bv))
    Wo = np.asarray(Wo, np.float32)
    bo = np.asarray(bo, np.float32)

    m = (np.arange(SQ)[None, :] >=
         (np.arange(4)[:, None, None] * 128 + np.arange(SK)[None, :, None])
         ).astype(BF16_NP)                       # [4, 128, 512]
    masks = np.concatenate([m, m], axis=2)       # [4, 128, 1024] (head pair)

    in_maps = []
    for c in range(N_CORES):
        b, hh = c // 2, c % 2
        hs = slice(hh * HPC, (hh + 1) * HPC)
        wo_c = Wo[hh * HPC * D:(hh + 1) * HPC * D]          # [512, E]
        # fold V-bias and half the output bias into one bias row
        bo2 = 0.5 * bo + bv[hs].reshape(HPC * D) @ wo_c
        in_maps.append({
            "XT": X[b].T.astype(BF16_NP),
            "Wq": Wq[hs].transpose(1, 0, 2).reshape(E, HPC * D).astype(BF16_NP),
            "Wk": Wk[hs].transpose(1, 0, 2).reshape(E, HPC * D).astype(BF16_NP),
            "Wv": Wv[hs].transpose(1, 0, 2).reshape(E, HPC * D).astype(BF16_NP),
            "bq": bq[hs].reshape(1, HPC * D).astype(BF16_NP),
            "bk": bk[hs].reshape(1, HPC * D).astype(BF16_NP),
            "Wo": wo_c.astype(BF16_NP),
            "bo2": bo2.astype(BF16_NP).reshape(1, E),
            "ones": np.ones((1, 128), np.float32),
            "onesb": np.ones((1, SQ), BF16_NP),
            "masks": np.ascontiguousarray(masks),
        })
    return in_maps


def assemble_output(results):
    # every core holds the full output for its batch; take the even core's
    return np.stack([results[2 * b]["out"] for b in range(B)])


def run(in_maps, **kw):
    nc = get_nc()
    return bass_utils.run_bass_kernel_spmd(nc, in_maps,
                                           core_ids=list(range(N_CORES)), **kw)


def kernel(X, Wq, Wk, Wv, bq, bk, bv, Wo, bo):
    in_maps = make_in_maps(X, Wq, Wk, Wv, bq, bk, bv, Wo, bo)
    res = run(in_maps)
    return assemble_output(res.results)


# revision 31
# speedup vs baseline: 1.0168x; 1.0168x over previous
"""Multi-head causal attention (B=4, S=2048, E=1024, H=16, D=64) on 8 TRN2 cores.

Sharding: core c handles batch c//2 and head-half c%2 (8 heads). Each core
computes Q/K/V projections, causal attention for its heads, and a partial
output projection over its heads; a 2-core bf16 AllGather per 256-token
chunk exchanges the pair's partials, and both cores sum them locally, so
every core holds the full output for its batch.

Layouts are transposed (feature-on-partition): the host supplies X^T and
head-packed weights so no on-chip transposes are needed. Attention runs in
S^T = K.Q^T layout (keys on partitions); softmax denominators come from a
ones-column appended to V so the PV matmul emits them for free. Scores for
a head pair go side by side into one 2-bank PSUM tile so a single ScalarE
exp covers both. The V bias and output bias are folded on the host into a
single bias row that enters the output projection as a K=1 matmul. Q/K
projections for later head pairs are interleaved with qt=0 attention so the
TensorE and ScalarE pipelines overlap from the start.
"""

import sys

sys.path.insert(0, "/opt/trn_rl_repo")

import numpy as np
import ml_dtypes

import concourse.bass as bass
import concourse.bacc as bacc
import concourse.tile as tile
import concourse.mybir as mybir
import concourse.bass_utils as bass_utils

B, S, E, H, D = 4, 2048, 1024, 16, 64
N_CORES = 8
HPC = H // 2          # heads per core
NPAIR = HPC // 2      # head pairs per core
SQ = 512              # q tile width
SK = 128              # k tile width
NQT = S // SQ         # 4
NKT = S // SK         # 16
NE = E // 128         # 8 contraction tiles
NCH = 8               # output exchange chunks (256 tokens each)
F32 = mybir.dt.float32
F32R = mybir.dt.float32r
BF16 = mybir.dt.bfloat16
BF16_NP = ml_dtypes.bfloat16
# bf16 CCE ReduceScatter faults the exec unit; AllGather is bypass (no CCE
# math) so bf16 works: gather both partials, then sum them locally.
CC_DT = mybir.dt.bfloat16

REPLICA_GROUPS = [[0, 1], [2, 3], [4, 5], [6, 7]]
AF = mybir.ActivationFunctionType
ALU = mybir.AluOpType


def build_kernel():
    nc = bacc.Bacc("TRN2", target_bir_lowering=False, debug=False,
                   num_devices=N_CORES)

    xt_d = nc.dram_tensor("XT", [E, S], BF16, kind="ExternalInput")
    wq_d = nc.dram_tensor("Wq", [E, HPC * D], BF16, kind="ExternalInput")
    wk_d = nc.dram_tensor("Wk", [E, HPC * D], BF16, kind="ExternalInput")
    wv_d = nc.dram_tensor("Wv", [E, HPC * D], BF16, kind="ExternalInput")
    bq_d = nc.dram_tensor("bq", [1, HPC * D], BF16, kind="ExternalInput")
    bk_d = nc.dram_tensor("bk", [1, HPC * D], BF16, kind="ExternalInput")
    wo_d = nc.dram_tensor("Wo", [HPC * D, E], BF16, kind="ExternalInput")
    bo2_d = nc.dram_tensor("bo2", [1, E], BF16, kind="ExternalInput")
    ones_d = nc.dram_tensor("ones", [1, 128], F32R, kind="ExternalInput")
    onesb_d = nc.dram_tensor("onesb", [1, SQ], BF16, kind="ExternalInput")
    mask_d = nc.dram_tensor("masks", [4, SK, 2 * SQ], BF16, kind="ExternalInput")
    out_d = nc.dram_tensor("out", [S, E], F32, kind="ExternalOutput")

    with tile.TileContext(nc) as tc:
        with (
            tc.tile_pool(name="persist", bufs=1) as persist,
            tc.tile_pool(name="dram", bufs=1, space="DRAM") as dram,
            tc.tile_pool(name="p1_in", bufs=1) as p1_in,
            tc.tile_pool(name="mm_ps", bufs=2, space="PSUM") as mm_ps,
            tc.tile_pool(name="st_ps", bufs=2, space="PSUM") as st_ps,
            tc.tile_pool(name="pv_ps", bufs=1, space="PSUM") as pv_ps,
            tc.tile_pool(name="probs", bufs=4) as probs_pool,
            tc.tile_pool(name="norm", bufs=2) as norm_pool,
            tc.tile_pool(name="op_sb", bufs=2) as op_sb,
        ):
            # ---- persistent SBUF tensors ----
            qt_sb = [persist.tile([128, S], BF16, tag=f"qt{p}", name=f"qt{p}")
                     for p in range(NPAIR)]
            kt_sb = [persist.tile([128, S], BF16, tag=f"kt{p}", name=f"kt{p}")
                     for p in range(NPAIR)]
            v_sb = [persist.tile([128, HPC, D + 1], BF16, tag=f"v{st}", name=f"v{st}")
                    for st in range(NKT)]
            ao_sb = [persist.tile([128, S], BF16, tag=f"ao{p}", name=f"ao{p}")
                     for p in range(NPAIR)]
            wo_sb = [persist.tile([128, E], BF16, tag=f"wo{p}", name=f"wo{p}")
                     for p in range(NPAIR)]
            bq_sb = persist.tile([1, HPC * D], BF16, tag="bq")
            bk_sb = persist.tile([1, HPC * D], BF16, tag="bk")
            bo2_sb = persist.tile([1, E], BF16, tag="bo2")
            ones_sb = persist.tile([1, 128], F32R, tag="ones")
            onesb_sb = persist.tile([1, SQ], BF16, tag="onesb")
            mask_sb = [persist.tile([SK, 2 * SQ], BF16, tag=f"mask{oi}",
                                    name=f"mask{oi}") for oi in range(4)]
            cc_in = [dram.tile([256, E], CC_DT, name=f"cc_in{c}")
                     for c in range(NCH)]
            cc_out = [dram.tile([512, E], CC_DT, name=f"cc_out{c}")
                      for c in range(NCH)]

            xt_sb = [p1_in.tile([128, S], BF16, tag=f"xt{e}", name=f"xt{e}")
                     for e in range(NE)]
            wq_sb = [p1_in.tile([128, HPC * D], BF16, tag=f"wq{e}", name=f"wq{e}")
                     for e in range(NE)]
            wk_sb = [p1_in.tile([128, HPC * D], BF16, tag=f"wk{e}", name=f"wk{e}")
                     for e in range(NE)]
            wv_sb = [p1_in.tile([128, HPC * D], BF16, tag=f"wv{e}", name=f"wv{e}")
                     for e in range(NE)]
            # DMA order = consumption order
            for e in range(NE):
                nc.sync.dma_start(xt_sb[e][:], xt_d[e * 128:(e + 1) * 128, :])
                nc.sync.dma_start(wv_sb[e][:], wv_d[e * 128:(e + 1) * 128, :])
                nc.sync.dma_start(wq_sb[e][:], wq_d[e * 128:(e + 1) * 128, :])
                nc.sync.dma_start(wk_sb[e][:], wk_d[e * 128:(e + 1) * 128, :])
            nc.sync.dma_start(bq_sb[:], bq_d[:])
            nc.sync.dma_start(bk_sb[:], bk_d[:])
            nc.sync.dma_start(bo2_sb[:], bo2_d[:])
            nc.sync.dma_start(ones_sb[:], ones_d[:])
            nc.sync.dma_start(onesb_sb[:], onesb_d[:])
            for oi in range(4):
                nc.sync.dma_start(mask_sb[oi][:], mask_d[oi])
            for p in range(NPAIR):
                nc.sync.dma_start(wo_sb[p][:], wo_d[p * 128:(p + 1) * 128, :])

            def emit_v(st):
                ps = mm_ps.tile([128, HPC * D], F32, tag="mm", name="psv")
                for e in range(NE):
                    nc.tensor.matmul(
                        ps[:],
                        xt_sb[e][:, st * 128:(st + 1) * 128],
                        wv_sb[e][:],
                        start=(e == 0), stop=(e == NE - 1),
                    )
                nc.vector.tensor_copy(
                    v_sb[st][:, :, 0:D],
                    ps[:].rearrange("p (h d) -> p h d", h=HPC),
                )
                nc.vector.memset(v_sb[st][:, :, D:D + 1], 1.0)

            def emit_qk(p):
                for st in range(NQT):
                    for w_sb, b_sb, o_sb in (
                        (wq_sb, bq_sb, qt_sb),
                        (wk_sb, bk_sb, kt_sb),
                    ):
                        ps = mm_ps.tile([128, SQ], F32, tag="mm", name="ps")
                        for e in range(NE):
                            nc.tensor.matmul(
                                ps[:],
                                w_sb[e][:, p * 128:(p + 1) * 128],
                                xt_sb[e][:, st * SQ:(st + 1) * SQ],
                                start=(e == 0), stop=False,
                            )
                        nc.tensor.matmul(  # + bias via K=1 ones row
                            ps[:], b_sb[0:1, p * 128:(p + 1) * 128],
                            onesb_sb[0:1, :], start=False, stop=True,
                        )
                        nc.vector.tensor_copy(o_sb[p][:, st * SQ:(st + 1) * SQ],
                                              ps[:])

            def emit_attn(qt, p):
                nkt_q = 4 * (qt + 1)  # causal: kt*128 <= qt*512+511
                qs = slice(qt * SQ, (qt + 1) * SQ)
                pv = [pv_ps.tile([128, SQ], F32, tag=f"pv{u}", name=f"pv{u}")
                      for u in range(2)]
                for kt in range(nkt_q):
                    ks = slice(kt * SK, (kt + 1) * SK)
                    sps = st_ps.tile([128, 2 * SQ], F32, tag="st", name="st")
                    for u in range(2):
                        nc.tensor.matmul(
                            sps[:, u * SQ:(u + 1) * SQ],
                            kt_sb[p][u * D:(u + 1) * D, ks],
                            qt_sb[p][u * D:(u + 1) * D, qs],
                            start=True, stop=True,
                        )
                    pt = probs_pool.tile([128, 2 * SQ], BF16, tag="pt", name="pt")
                    oi = kt - 4 * qt
                    if oi >= 0:  # diagonal block: exp then mask both halves
                        et = probs_pool.tile([128, 2 * SQ], BF16, tag="et",
                                             name="et")
                        nc.scalar.activation(et[:], sps[:], AF.Exp, scale=0.125)
                        nc.vector.tensor_tensor(pt[:], et[:], mask_sb[oi][:],
                                                ALU.mult)
                    else:
                        nc.scalar.activation(pt[:], sps[:], AF.Exp, scale=0.125)
                    for u in range(2):
                        nc.tensor.matmul(
                            pv[u][0:D + 1, :],
                            v_sb[kt][:, 2 * p + u, :],
                            pt[:, u * SQ:(u + 1) * SQ],
                            start=(kt == 0), stop=(kt == nkt_q - 1),
                        )
                # copy out + denom to SBUF first, freeing both pv banks early
                pvs, den = [], []
                for u in range(2):
                    pvs.append(norm_pool.tile([D, SQ], F32, tag=f"pvs{u}",
                                              name=f"pvs{u}"))
                    nc.vector.tensor_copy(pvs[u][:], pv[u][0:D, :])
                    den.append(norm_pool.tile([1, SQ], F32, tag=f"den{u}",
                                              name=f"den{u}"))
                    nc.vector.tensor_copy(den[u][:], pv[u][D:D + 1, :])
                for u in range(2):
                    lo, hi = u * D, (u + 1) * D
                    rcp = norm_pool.tile([1, SQ], F32, tag="rcp", name="rcp")
                    nc.vector.reciprocal_approx_fast(rcp[:], den[u][:])
                    rcp_r = norm_pool.tile([1, SQ], F32R, tag="rcp_r", name="rcp_r")
                    nc.vector.tensor_copy(rcp_r[:], rcp[:])
                    # broadcast 1/den across the d partitions
                    bc = mm_ps.tile([D, SQ], F32, tag="mm", name=f"bc{u}")
                    nc.tensor.matmul(bc[:], ones_sb[0:1, 0:D], rcp_r[:],
                                     start=True, stop=True)
                    nc.vector.tensor_tensor(
                        ao_sb[p][lo:hi, qs], pvs[u][:], bc[:], ALU.mult,
                    )

            def emit_chunk(cq):
                # output projection + exchange for 256 tokens
                for sst in range(2):
                    stg = cq * 2 + sst
                    ss = slice(stg * 128, (stg + 1) * 128)
                    for et in range(2):
                        es = slice(et * 512, (et + 1) * 512)
                        ps = mm_ps.tile([128, 512], F32, tag="mm", name="op")
                        for p2 in range(NPAIR):
                            nc.tensor.matmul(
                                ps[:], ao_sb[p2][:, ss], wo_sb[p2][:, es],
                                start=(p2 == 0), stop=False,
                            )
                        nc.tensor.matmul(  # + (bo/2 + bv@Wo) via K=1 ones row
                            ps[:], onesb_sb[0:1, 0:128], bo2_sb[0:1, es],
                            start=False, stop=True,
                        )
                        ob = op_sb.tile([128, 512], CC_DT, tag="ob", name="ob")
                        nc.vector.tensor_copy(ob[:], ps[:])
                        nc.sync.dma_start(cc_in[cq][sst * 128:(sst + 1) * 128, es],
                                          ob[:])
                nc.gpsimd.collective_compute(
                    "AllGather", ALU.bypass,
                    ins=[cc_in[cq][:].opt()],
                    outs=[cc_out[cq][:].opt()],
                    replica_groups=REPLICA_GROUPS,
                )
                # rows 0-255 = rank0's partial, 256-511 = rank1's; both cores
                # materialize the full sum for the 256 tokens.
                for sst2 in range(2):
                    ra = op_sb.tile([128, E], CC_DT, tag="ra", name="ra")
                    rb = op_sb.tile([128, E], CC_DT, tag="rb", name="rb")
                    nc.sync.dma_start(
                        ra[:], cc_out[cq][sst2 * 128:(sst2 + 1) * 128, :])
                    nc.sync.dma_start(
                        rb[:], cc_out[cq][256 + sst2 * 128:256 + (sst2 + 1) * 128, :])
                    sm = op_sb.tile([128, E], F32, tag="sm", name="sm")
                    nc.vector.tensor_tensor(sm[:], ra[:], rb[:], ALU.add)
                    nc.sync.dma_start(
                        out_d[cq * 256 + sst2 * 128:cq * 256 + (sst2 + 1) * 128, :],
                        sm[:])

            # ---- schedule ----
            for st in range(4):
                emit_v(st)
            emit_qk(0)
            for st in range(4, NKT):
                emit_v(st)
            # interleave remaining projections with qt=0 attention
            for p in range(NPAIR):
                if p + 1 < NPAIR:
                    emit_qk(p + 1)
                emit_attn(0, p)
            emit_chunk(0)
            emit_chunk(1)
            for qt in range(1, NQT):
                for p in range(NPAIR):
                    emit_attn(qt, p)
                emit_chunk(2 * qt)
                emit_chunk(2 * qt + 1)

    nc.compile()
    return nc


_NC_CACHE = None


def get_nc():
    global _NC_CACHE
    if _NC_CACHE is None:
        _NC_CACHE = build_kernel()
    return _NC_CACHE


def make_in_maps(X, Wq, Wk, Wv, bq, bk, bv, Wo, bo):
    X = np.asarray(X, np.float32)
    Wq, Wk, Wv = (np.asarray(w, np.float32) for w in (Wq, Wk, Wv))
    bq, bk, bv = (np.asarray(b, np.float32) for b in (bq, bk, bv))
    Wo = np.asarray(Wo, np.float32)
    bo = np.asarray(bo, np.float32)

    m = (np.arange(SQ)[None, :] >=
         (np.arange(4)[:, None, None] * 128 + np.arange(SK)[None, :, None])
         ).astype(BF16_NP)                       # [4, 128, 512]
    masks = np.concatenate([m, m], axis=2)       # [4, 128, 1024] (head pair)

    in_maps = []
    for c in range(N_CORES):
        b, hh = c // 2, c % 2
        hs = slice(hh * HPC, (hh + 1) * HPC)
        wo_c = Wo[hh * HPC * D:(hh + 1) * HPC * D]          # [512, E]
        # fold V-bias and half the output bias into one bias row
        bo2 = 0.5 * bo + bv[hs].reshape(HPC * D) @ wo_c
        in_maps.append({
            "XT": X[b].T.astype(BF16_NP),
            "Wq": Wq[hs].transpose(1, 0, 2).reshape(E, HPC * D).astype(BF16_NP),
            "Wk": Wk[hs].transpose(1, 0, 2).reshape(E, HPC * D).astype(BF16_NP),
            "Wv": Wv[hs].transpose(1, 0, 2).reshape(E, HPC * D).astype(BF16_NP),
            "bq": bq[hs].reshape(1, HPC * D).astype(BF16_NP),
            "bk": bk[hs].reshape(1, HPC * D).astype(BF16_NP),
            "Wo": wo_c.astype(BF16_NP),
            "bo2": bo2.astype(BF16_NP).reshape(1, E),
            "ones": np.ones((1, 128), np.float32),
            "onesb": np.ones((1, SQ), BF16_NP),
            "masks": np.ascontiguousarray(masks),
        })
    return in_maps


def assemble_output(results):
    # every core holds the full output for its batch; take the even core's
    return np.stack([results[2 * b]["out"] for b in range(B)])


def run(in_maps, **kw):
    nc = get_nc()
    return bass_utils.run_bass_kernel_spmd(nc, in_maps,
                                           core_ids=list(range(N_CORES)), **kw)


def kernel(X, Wq, Wk, Wv, bq, bk, bv, Wo, bo):
    in_maps = make_in_maps(X, Wq, Wk, Wv, bq, bk, bv, Wo, bo)
    res = run(in_maps)
    return assemble_output(res.results)
